# revision 4
# baseline (speedup 1.0000x reference)
"""IF spiking-neuron scan (charge / fire / hard-reset) on 8 Trainium2 cores.

Reference recurrence over t (elementwise on every [B, N] element):
    v = v + x_t
    s = (v - 1.0 >= 0)          # spike, 0.0/1.0
    v = (1 - s) * v             # hard reset to 0

Sharding: pure data parallel over the B*N = 262144 element dimension;
each of the 8 cores owns 32768 element chains with zero communication.
Per core the chains live in SBUF as a [128, 256] f32 state tile; the
64-step scan runs locally, bit-exact vs the reference.

v2 design (measured on HW):
  - The serial 64-step chain on the DVE is the wall: 2 ops/step
    (tensor_tensor add + fused scalar_tensor_tensor reset) in two
    interleaved half-width streams paces at ~850 ns/step; all other
    structures measured worse (Pool-engine offload, single full-width
    stream, cross-engine ping-pong).
  - Input x is pre-transposed on the host to [P, T, F] per core, so a
    timestep block is ONE contiguous multi-KiB descriptor per partition
    (the old [T, P*F] layout forced 1 KiB descriptors at ring rate).
    Blocks alternate between the SP and ACT hardware DGE rings
    (~205 + ~230 GB/s measured) so input always stays ahead of the
    chain and block 0 lands ASAP.
  - Spikes are ONE ACT pass per block: z = Sign(V_TH - u) written
    directly as uint8. The float->uint8 conversion saturates negatives
    to 0 (verified on HW), so z = [u < V_TH] = 1 - s exactly, including
    u == V_TH (Sign(0) = 0 -> spike). The host flips it back.
  - Whole input + u history stay resident in SBUF (96 KiB/partition of
    208) - pools never recycle, so no WAR waits on the chain.
  - Small blocks at both ends shrink pipeline fill (first input chunk)
    and drain (last ACT pass + output DMA).
"""

import numpy as np

import concourse.tile as tile
from concourse import bacc, mybir
from concourse.bass_utils import run_bass_kernel_spmd

T = 64
B = 32
N = 8192
NCORES = 8
PERCORE = (B * N) // NCORES  # 32768 element chains per core
P = 128                      # SBUF partitions
F = PERCORE // P             # 256 elements per partition
H = F // 2                   # half-width for the two interleaved streams

V_TH = 1.0

# timestep block sizes: small at the edges to cut pipeline fill/drain
BLOCKS = [2, 2, 4] + [8] * 6 + [4, 2, 2]
assert sum(BLOCKS) == T

_NC_CACHE = {}


def build_nc(blocks=None):
    blocks = list(BLOCKS if blocks is None else blocks)
    # Bacc (not raw Bass): its compile() splits multi-wait sync conditions
    # into nop/event-semaphore prefixes — walrus accepts at most one sync
    # wait per hardware instruction.
    nc = bacc.Bacc("TRN2", target_bir_lowering=False, debug=False)
    x = nc.dram_tensor("x", [P, T, F], mybir.dt.float32, kind="ExternalInput").ap()
    y = nc.dram_tensor("y", [P, T, F], mybir.dt.uint8, kind="ExternalOutput").ap()

    with tile.TileContext(nc) as tc:
        with (
            tc.tile_pool(name="xin", bufs=len(blocks)) as xpool,
            tc.tile_pool(name="sout", bufs=4) as spool,
            tc.tile_pool(name="ub", bufs=len(blocks)) as ubpool,
            tc.tile_pool(name="v", bufs=1) as vpool,
        ):
            v = vpool.tile([P, F], mybir.dt.float32)
            nc.vector.memset(v[:], 0.0)
            # Issue ALL input dma_starts first: input tiles have no deps, so
            # both DGE rings fill their descriptor queues up front and stream
            # ahead of the chain. (Interleaving them with the per-block ACT
            # ops would serialize each ring's next input chunk behind the
            # previous block's activation - the sequencers are in-order.)
            xts = []
            t0 = 0
            for bi, tb in enumerate(blocks):
                xt = xpool.tile([P, tb * F], mybir.dt.float32, tag="xin")
                if t0 == 0 and tb > 1:
                    # Block 0 gates the whole scan: split it across both
                    # HW-DGE rings so the first timestep lands sooner.
                    nc.sync.dma_start(xt[:, :F], x[:, 0:1, :])
                    nc.scalar.dma_start(xt[:, F:], x[:, 1:tb, :])
                else:
                    eng = nc.sync if bi % 2 == 1 else nc.scalar
                    eng.dma_start(xt[:], x[:, t0:t0 + tb, :])
                xts.append(xt)
                t0 += tb
            t0 = 0
            for bi, tb in enumerate(blocks):
                xt = xts[bi]
                ub = ubpool.tile([P, tb * F], mybir.dt.float32, tag="ub")
                for ti in range(tb):
                    for h in range(2):
                        lo = ti * F + h * H
                        nc.vector.tensor_add(
                            ub[:, lo:lo + H], v[:, h * H:(h + 1) * H],
                            xt[:, lo:lo + H],
                        )
                    if t0 + ti == T - 1:
                        continue  # v after the final timestep is never read
                    for h in range(2):
                        lo = ti * F + h * H
                        nc.vector.scalar_tensor_tensor(
                            v[:, h * H:(h + 1) * H], ub[:, lo:lo + H], V_TH,
                            ub[:, lo:lo + H],
                            mybir.AluOpType.is_lt, mybir.AluOpType.mult,
                        )
                st = spool.tile([P, tb * F], mybir.dt.uint8, tag="sout")
                # One ACT pass: z = Sign(V_TH - u) in {-1,0,1}; the uint8
                # store saturates to {0,1}, giving z = [u < V_TH] = 1 - s
                # exactly (u == V_TH -> Sign(0) = 0 -> spike). Host flips.
                nc.scalar.activation(
                    st[:], ub[:], mybir.ActivationFunctionType.Sign,
                    bias=V_TH, scale=-1.0,
                )
                # outputs ride the sync ring; they enqueue after all its
                # input chunks (FIFO), which are long done by then
                nc.sync.dma_start(y[:, t0:t0 + tb, :], st[:])
                t0 += tb
    nc.compile()
    return nc


def _get_nc():
    if "nc" not in _NC_CACHE:
        _NC_CACHE["nc"] = build_nc()
    return _NC_CACHE["nc"]


def run_sharded(x_seq, trace=False, nc=None, **kwargs):
    if nc is None:
        nc = _get_nc()
    x2 = np.asarray(x_seq, dtype=np.float32).reshape(T, B * N)
    in_maps = []
    for c in range(NCORES):
        # core slab [T, PERCORE] -> [P, T, F]: partition-major, time
        # contiguous per partition so each block is one fat descriptor
        xc = x2[:, c * PERCORE:(c + 1) * PERCORE].reshape(T, P, F)
        in_maps.append({"x": np.ascontiguousarray(xc.transpose(1, 0, 2))})
    # A cold device occasionally reports NRT_EXEC_UNIT_UNRECOVERABLE on the
    # first execute and recovers on the next attempt; retry a couple times.
    for attempt in range(3):
        try:
            res = run_bass_kernel_spmd(
                nc, in_maps, list(range(NCORES)), trace=trace, **kwargs
            )
            break
        except Exception:  # jax.errors.JaxRuntimeError and friends
            if attempt == 2:
                raise
            import time
            time.sleep(2.0)
    out = np.empty((T, B * N), dtype=np.float32)
    for c in range(NCORES):
        zc = np.asarray(res.results[c]["y"])          # [P, T, F] uint8, z = 1-s
        r = zc.transpose(1, 0, 2).reshape(T, PERCORE)
        out[:, c * PERCORE:(c + 1) * PERCORE] = 1 - r
    return out.reshape(T, B, N), res


def kernel(x_seq):
    out, _ = run_sharded(x_seq)
    return out


# revision 8
# speedup vs baseline: 1.9018x; 1.9018x over previous
"""IF spiking-neuron scan (charge / fire / hard-reset) on 8 Trainium2 cores.

Reference recurrence over t (elementwise on every [B, N] element):
    v = v + x_t
    s = (v - 1.0 >= 0)          # spike, 0.0/1.0
    v = (1 - s) * v             # hard reset to 0

Sharding: pure data parallel over the B*N = 262144 element dimension;
each of the 8 cores owns 32768 element chains with zero communication.
Per core the chains live in SBUF as a [128, 256] f32 state tile; the
64-step scan runs locally, bit-exact vs the reference.

v2 design (measured on HW):
  - The serial 64-step chain on the DVE is the wall: 2 ops/step
    (tensor_tensor add + fused scalar_tensor_tensor reset) in two
    interleaved half-width streams paces at ~850 ns/step; all other
    structures measured worse (Pool-engine offload, single full-width
    stream, cross-engine ping-pong).
  - Input x is pre-transposed on the host to [P, T, F] per core, so a
    timestep block is ONE contiguous multi-KiB descriptor per partition
    (the old [T, P*F] layout forced 1 KiB descriptors at ring rate).
    Blocks alternate between the SP and ACT hardware DGE rings
    (~205 + ~230 GB/s measured) so input always stays ahead of the
    chain and block 0 lands ASAP.
  - Spikes are ONE ACT pass per block: z = Sign(V_TH - u) written
    directly as uint8. The float->uint8 conversion saturates negatives
    to 0 (verified on HW), so z = [u < V_TH] = 1 - s exactly, including
    u == V_TH (Sign(0) = 0 -> spike). The host flips it back.
  - Whole input + u history stay resident in SBUF (96 KiB/partition of
    208) - pools never recycle, so no WAR waits on the chain.
  - Small blocks at both ends shrink pipeline fill (first input chunk)
    and drain (last ACT pass + output DMA).
"""

import numpy as np

import concourse.tile as tile
from concourse import bacc, mybir
from concourse.bass_utils import run_bass_kernel_spmd

T = 64
B = 32
N = 8192
NCORES = 8
PERCORE = (B * N) // NCORES  # 32768 element chains per core
P = 128                      # SBUF partitions
F = PERCORE // P             # 256 elements per partition
H = F // 2                   # half-width for the two interleaved streams

V_TH = 1.0

# timestep block sizes: small at the edges to cut pipeline fill/drain
# (block 0 is a single step: u_0 = x_0, so the chain starts the moment
# one timestep lands; the 1-step final blocks shrink the ACT+DMA drain)
BLOCKS = [1, 2, 5] + [8] * 6 + [4, 2, 1, 1]
assert sum(BLOCKS) == T

_NC_CACHE = {}


def build_nc(blocks=None):
    blocks = list(BLOCKS if blocks is None else blocks)
    # Bacc (not raw Bass): its compile() splits multi-wait sync conditions
    # into nop/event-semaphore prefixes — walrus accepts at most one sync
    # wait per hardware instruction.
    nc = bacc.Bacc("TRN2", target_bir_lowering=False, debug=False)
    x = nc.dram_tensor("x", [P, T, F], mybir.dt.float32, kind="ExternalInput").ap()
    y = nc.dram_tensor("y", [P, T, F], mybir.dt.uint8, kind="ExternalOutput").ap()

    with tile.TileContext(nc) as tc:
        with (
            tc.tile_pool(name="xin", bufs=len(blocks)) as xpool,
            tc.tile_pool(name="sout", bufs=4) as spool,
            tc.tile_pool(name="ub", bufs=4) as ubpool,
            tc.tile_pool(name="v", bufs=1) as vpool,
        ):
            v = vpool.tile([P, F], mybir.dt.float32)
            nc.vector.memset(v[:], 0.0)
            # Issue ALL input dma_starts first: input tiles have no deps, so
            # both DGE rings fill their descriptor queues up front and stream
            # ahead of the chain. (Interleaving them with the per-block ACT
            # ops would serialize each ring's next input chunk behind the
            # previous block's activation - the sequencers are in-order.)
            xts = []
            t0 = 0
            for bi, tb in enumerate(blocks):
                xt = xpool.tile([P, tb * F], mybir.dt.float32, tag="xin")
                if t0 == 0 and tb > 1:
                    # Block 0 gates the whole scan: split it across both
                    # HW-DGE rings so the first timestep lands sooner.
                    nc.sync.dma_start(xt[:, :F], x[:, 0:1, :])
                    nc.scalar.dma_start(xt[:, F:], x[:, 1:tb, :])
                else:
                    eng = nc.sync if bi % 2 == 1 else nc.scalar
                    eng.dma_start(xt[:], x[:, t0:t0 + tb, :])
                xts.append(xt)
                t0 += tb
            t0 = 0
            for bi, tb in enumerate(blocks):
                xt = xts[bi]
                ub = ubpool.tile([P, tb * F], mybir.dt.float32, tag="ub")
                for ti in range(tb):
                    if t0 + ti == 0:
                        # v_0 = 0, so u_0 = x_0: skip the add, the reset and
                        # the spike pass read the x tile directly
                        for h in range(2):
                            nc.vector.scalar_tensor_tensor(
                                v[:, h * H:(h + 1) * H], xt[:, h * H:h * H + H],
                                V_TH, xt[:, h * H:h * H + H],
                                mybir.AluOpType.is_lt, mybir.AluOpType.mult,
                            )
                        continue
                    for h in range(2):
                        lo = ti * F + h * H
                        nc.vector.tensor_add(
                            ub[:, lo:lo + H], v[:, h * H:(h + 1) * H],
                            xt[:, lo:lo + H],
                        )
                    if t0 + ti == T - 1:
                        continue  # v after the final timestep is never read
                    for h in range(2):
                        lo = ti * F + h * H
                        nc.vector.scalar_tensor_tensor(
                            v[:, h * H:(h + 1) * H], ub[:, lo:lo + H], V_TH,
                            ub[:, lo:lo + H],
                            mybir.AluOpType.is_lt, mybir.AluOpType.mult,
                        )
                st = spool.tile([P, tb * F], mybir.dt.uint8, tag="sout")
                # One ACT pass: z = Sign(V_TH - u) in {-1,0,1}; the uint8
                # store saturates to {0,1}, giving z = [u < V_TH] = 1 - s
                # exactly (u == V_TH -> Sign(0) = 0 -> spike). Host flips.
                usrc = xt if t0 == 0 and tb == 1 else ub
                nc.scalar.activation(
                    st[:], usrc[:], mybir.ActivationFunctionType.Sign,
                    bias=V_TH, scale=-1.0,
                )
                # outputs ride the sync ring; they enqueue after all its
                # input chunks (FIFO), which are long done by then
                nc.sync.dma_start(y[:, t0:t0 + tb, :], st[:])
                t0 += tb
    nc.compile()
    return nc


def _get_nc():
    if "nc" not in _NC_CACHE:
        _NC_CACHE["nc"] = build_nc()
    return _NC_CACHE["nc"]


def run_sharded(x_seq, trace=False, nc=None, **kwargs):
    if nc is None:
        nc = _get_nc()
    x2 = np.asarray(x_seq, dtype=np.float32).reshape(T, B * N)
    in_maps = []
    for c in range(NCORES):
        # core slab [T, PERCORE] -> [P, T, F]: partition-major, time
        # contiguous per partition so each block is one fat descriptor
        xc = x2[:, c * PERCORE:(c + 1) * PERCORE].reshape(T, P, F)
        in_maps.append({"x": np.ascontiguousarray(xc.transpose(1, 0, 2))})
    # A cold device occasionally reports NRT_EXEC_UNIT_UNRECOVERABLE on the
    # first execute and recovers on the next attempt; retry a couple times.
    for attempt in range(3):
        try:
            res = run_bass_kernel_spmd(
                nc, in_maps, list(range(NCORES)), trace=trace, **kwargs
            )
            break
        except Exception:  # jax.errors.JaxRuntimeError and friends
            if attempt == 2:
                raise
            import time
            time.sleep(2.0)
    out = np.empty((T, B * N), dtype=np.float32)
    for c in range(NCORES):
        zc = np.asarray(res.results[c]["y"])          # [P, T, F] uint8, z = 1-s
        r = zc.transpose(1, 0, 2).reshape(T, PERCORE)
        out[:, c * PERCORE:(c + 1) * PERCORE] = 1 - r
    return out.reshape(T, B, N), res


def kernel(x_seq):
    out, _ = run_sharded(x_seq)
    return out


# revision 10
# speedup vs baseline: 1.9872x; 1.0449x over previous
"""IF spiking-neuron scan (charge / fire / hard-reset) on 8 Trainium2 cores.

Reference recurrence over t (elementwise on every [B, N] element):
    v = v + x_t
    s = (v - 1.0 >= 0)          # spike, 0.0/1.0
    v = (1 - s) * v             # hard reset to 0

Sharding: pure data parallel over the B*N = 262144 element dimension;
each of the 8 cores owns 32768 element chains with zero communication.
Per core the chains live in SBUF as a [128, 256] f32 state tile; the
64-step scan runs locally, bit-exact vs the reference.

v2 design (measured on HW):
  - The serial 64-step chain on the DVE is the wall: 2 ops/step
    (tensor_tensor add + fused scalar_tensor_tensor reset) in two
    interleaved half-width streams paces at ~850 ns/step; all other
    structures measured worse (Pool-engine offload, single full-width
    stream, cross-engine ping-pong).
  - Input x is pre-transposed on the host to [P, T, F] per core, so a
    timestep block is ONE contiguous multi-KiB descriptor per partition
    (the old [T, P*F] layout forced 1 KiB descriptors at ring rate).
    Blocks alternate between the SP and ACT hardware DGE rings
    (~205 + ~230 GB/s measured) so input always stays ahead of the
    chain and block 0 lands ASAP.
  - Spikes are ONE ACT pass per block: z = Sign(V_TH - u) written
    directly as uint8. The float->uint8 conversion saturates negatives
    to 0 (verified on HW), so z = [u < V_TH] = 1 - s exactly, including
    u == V_TH (Sign(0) = 0 -> spike). The host flips it back.
  - Whole input + u history stay resident in SBUF (96 KiB/partition of
    208) - pools never recycle, so no WAR waits on the chain.
  - Small blocks at both ends shrink pipeline fill (first input chunk)
    and drain (last ACT pass + output DMA).
"""

import numpy as np

import concourse.tile as tile
from concourse import bacc, mybir
from concourse.bass_utils import run_bass_kernel_spmd

T = 64
B = 32
N = 8192
NCORES = 8
PERCORE = (B * N) // NCORES  # 32768 element chains per core
P = 128                      # SBUF partitions
F = PERCORE // P             # 256 elements per partition
H = F // 2                   # half-width for the two interleaved streams

V_TH = 1.0

# timestep block sizes: small at the edges to cut pipeline fill/drain
# (block 0 is a single step: u_0 = x_0, so the chain starts the moment
# one timestep lands; the 1-step final blocks shrink the ACT+DMA drain)
BLOCKS = [1, 2, 5] + [8] * 6 + [4, 2, 1, 1]
assert sum(BLOCKS) == T

_NC_CACHE = {}


def build_nc(blocks=None):
    blocks = list(BLOCKS if blocks is None else blocks)
    # Bacc (not raw Bass): its compile() splits multi-wait sync conditions
    # into nop/event-semaphore prefixes — walrus accepts at most one sync
    # wait per hardware instruction.
    nc = bacc.Bacc("TRN2", target_bir_lowering=False, debug=False)
    x = nc.dram_tensor("x", [P, T, F], mybir.dt.float32, kind="ExternalInput").ap()
    y = nc.dram_tensor("y", [P, T, F], mybir.dt.uint8, kind="ExternalOutput").ap()

    with tile.TileContext(nc) as tc:
        with (
            tc.tile_pool(name="xin", bufs=len(blocks)) as xpool,
            tc.tile_pool(name="sout", bufs=4) as spool,
            tc.tile_pool(name="ub", bufs=4) as ubpool,
            tc.tile_pool(name="v", bufs=1) as vpool,
        ):
            v = vpool.tile([P, F], mybir.dt.float32)
            nc.vector.memset(v[:], 0.0)
            # Issue ALL input dma_starts first: input tiles have no deps, so
            # both DGE rings fill their descriptor queues up front and stream
            # ahead of the chain. (Interleaving them with the per-block ACT
            # ops would serialize each ring's next input chunk behind the
            # previous block's activation - the sequencers are in-order.)
            xts = []
            t0 = 0
            for bi, tb in enumerate(blocks):
                xt = xpool.tile([P, tb * F], mybir.dt.float32, tag="xin")
                # split EVERY block across both HW-DGE rings: each block
                # lands in half the time and the early blocks never gate
                # the chain (ring rate drops to ~170 GB/s once the DVE is
                # streaming, so a single ring can fall behind at the start)
                if tb == 1:
                    hp = P // 2
                    nc.sync.dma_start(xt[:hp, :], x[:hp, t0:t0 + 1, :])
                    nc.scalar.dma_start(xt[hp:, :], x[hp:, t0:t0 + 1, :])
                else:
                    th = (tb + 1) // 2
                    nc.sync.dma_start(xt[:, :th * F], x[:, t0:t0 + th, :])
                    nc.scalar.dma_start(xt[:, th * F:], x[:, t0 + th:t0 + tb, :])
                xts.append(xt)
                t0 += tb
            t0 = 0
            for bi, tb in enumerate(blocks):
                xt = xts[bi]
                ub = ubpool.tile([P, tb * F], mybir.dt.float32, tag="ub")
                for ti in range(tb):
                    if t0 + ti == 0:
                        # v_0 = 0, so u_0 = x_0: skip the add, the reset and
                        # the spike pass read the x tile directly
                        for h in range(2):
                            nc.vector.scalar_tensor_tensor(
                                v[:, h * H:(h + 1) * H], xt[:, h * H:h * H + H],
                                V_TH, xt[:, h * H:h * H + H],
                                mybir.AluOpType.is_lt, mybir.AluOpType.mult,
                            )
                        continue
                    for h in range(2):
                        lo = ti * F + h * H
                        nc.vector.tensor_add(
                            ub[:, lo:lo + H], v[:, h * H:(h + 1) * H],
                            xt[:, lo:lo + H],
                        )
                    if t0 + ti == T - 1:
                        continue  # v after the final timestep is never read
                    for h in range(2):
                        lo = ti * F + h * H
                        nc.vector.scalar_tensor_tensor(
                            v[:, h * H:(h + 1) * H], ub[:, lo:lo + H], V_TH,
                            ub[:, lo:lo + H],
                            mybir.AluOpType.is_lt, mybir.AluOpType.mult,
                        )
                st = spool.tile([P, tb * F], mybir.dt.uint8, tag="sout")
                usrc = xt if t0 == 0 and tb == 1 else ub
                if t0 + tb > T - 3:
                    # final blocks: the ACT hop (engine handoff + 222-cycle
                    # SBUF latency) is a pure tail; one DVE op computes
                    # z = (u < V_TH) directly instead
                    nc.vector.tensor_scalar(
                        st[:], usrc[:], V_TH, None, mybir.AluOpType.is_lt
                    )
                else:
                    # One ACT pass: z = Sign(V_TH - u) in {-1,0,1}; the uint8
                    # store saturates to {0,1}, so z = [u < V_TH] = 1 - s
                    # exactly (u == V_TH -> Sign(0) = 0 -> spike). Host flips.
                    nc.scalar.activation(
                        st[:], usrc[:], mybir.ActivationFunctionType.Sign,
                        bias=V_TH, scale=-1.0,
                    )
                # outputs alternate rings; they enqueue behind that ring's
                # input chunks (FIFO), which are long done by then
                oeng = nc.sync if bi % 2 == 0 else nc.scalar
                oeng.dma_start(y[:, t0:t0 + tb, :], st[:])
                t0 += tb
    nc.compile()
    return nc


def _get_nc():
    if "nc" not in _NC_CACHE:
        _NC_CACHE["nc"] = build_nc()
    return _NC_CACHE["nc"]


def run_sharded(x_seq, trace=False, nc=None, **kwargs):
    if nc is None:
        nc = _get_nc()
    x2 = np.asarray(x_seq, dtype=np.float32).reshape(T, B * N)
    in_maps = []
    for c in range(NCORES):
        # core slab [T, PERCORE] -> [P, T, F]: partition-major, time
        # contiguous per partition so each block is one fat descriptor
        xc = x2[:, c * PERCORE:(c + 1) * PERCORE].reshape(T, P, F)
        in_maps.append({"x": np.ascontiguousarray(xc.transpose(1, 0, 2))})
    # A cold device occasionally reports NRT_EXEC_UNIT_UNRECOVERABLE on the
    # first execute and recovers on the next attempt; retry a couple times.
    for attempt in range(3):
        try:
            res = run_bass_kernel_spmd(
                nc, in_maps, list(range(NCORES)), trace=trace, **kwargs
            )
            break
        except Exception:  # jax.errors.JaxRuntimeError and friends
            if attempt == 2:
                raise
            import time
            time.sleep(2.0)
    out = np.empty((T, B * N), dtype=np.float32)
    for c in range(NCORES):
        zc = np.asarray(res.results[c]["y"])          # [P, T, F] uint8, z = 1-s
        r = zc.transpose(1, 0, 2).reshape(T, PERCORE)
        out[:, c * PERCORE:(c + 1) * PERCORE] = 1 - r
    return out.reshape(T, B, N), res


def kernel(x_seq):
    out, _ = run_sharded(x_seq)
    return out


# revision 12
# speedup vs baseline: 2.0677x; 1.0405x over previous
"""IF spiking-neuron scan (charge / fire / hard-reset) on 8 Trainium2 cores.

Reference recurrence over t (elementwise on every [B, N] element):
    v = v + x_t
    s = (v - 1.0 >= 0)          # spike, 0.0/1.0
    v = (1 - s) * v             # hard reset to 0

Sharding: pure data parallel over the B*N = 262144 element dimension;
each of the 8 cores owns 32768 element chains with zero communication.
Per core the chains live in SBUF as a [128, 256] f32 state tile; the
64-step scan runs locally, bit-exact vs the reference.

v2 design (measured on HW):
  - The serial 64-step chain on the DVE is the wall: 2 ops/step
    (tensor_tensor add + fused scalar_tensor_tensor reset) in two
    interleaved half-width streams paces at ~850 ns/step; all other
    structures measured worse (Pool-engine offload, single full-width
    stream, cross-engine ping-pong).
  - Input x is pre-transposed on the host to [P, T, F] per core, so a
    timestep block is ONE contiguous multi-KiB descriptor per partition
    (the old [T, P*F] layout forced 1 KiB descriptors at ring rate).
    Blocks alternate between the SP and ACT hardware DGE rings
    (~205 + ~230 GB/s measured) so input always stays ahead of the
    chain and block 0 lands ASAP.
  - Spikes are ONE ACT pass per block: z = Sign(V_TH - u) written
    directly as uint8. The float->uint8 conversion saturates negatives
    to 0 (verified on HW), so z = [u < V_TH] = 1 - s exactly, including
    u == V_TH (Sign(0) = 0 -> spike). The host flips it back.
  - Whole input + u history stay resident in SBUF (96 KiB/partition of
    208) - pools never recycle, so no WAR waits on the chain.
  - Small blocks at both ends shrink pipeline fill (first input chunk)
    and drain (last ACT pass + output DMA).
"""

import numpy as np

import concourse.tile as tile
from concourse import bacc, mybir
from concourse.bass_utils import run_bass_kernel_spmd

T = 64
B = 32
N = 8192
NCORES = 8
PERCORE = (B * N) // NCORES  # 32768 element chains per core
P = 128                      # SBUF partitions
F = PERCORE // P             # 256 elements per partition
H = F // 2                   # half-width for the two interleaved streams

V_TH = 1.0

# timestep block sizes: small at the edges to cut pipeline fill/drain
# (block 0 is a single step: u_0 = x_0, so the chain starts the moment
# one timestep lands; the 1-step final blocks shrink the ACT+DMA drain)
BLOCKS = [1, 2, 3, 4, 6] + [8] * 5 + [4, 2, 1, 1]
assert sum(BLOCKS) == T

_NC_CACHE = {}


def build_nc(blocks=None):
    blocks = list(BLOCKS if blocks is None else blocks)
    # Bacc (not raw Bass): its compile() splits multi-wait sync conditions
    # into nop/event-semaphore prefixes — walrus accepts at most one sync
    # wait per hardware instruction.
    nc = bacc.Bacc("TRN2", target_bir_lowering=False, debug=False)
    x = nc.dram_tensor("x", [P, T, F], mybir.dt.float32, kind="ExternalInput").ap()
    y = nc.dram_tensor("y", [P, T, F], mybir.dt.uint8, kind="ExternalOutput").ap()

    with tile.TileContext(nc) as tc:
        with (
            tc.tile_pool(name="xin", bufs=len(blocks)) as xpool,
            tc.tile_pool(name="sout", bufs=4) as spool,
            tc.tile_pool(name="ub", bufs=6) as ubpool,
            tc.tile_pool(name="v", bufs=1) as vpool,
        ):
            v = vpool.tile([P, F], mybir.dt.float32)
            nc.vector.memset(v[:], 0.0)
            # Issue ALL input dma_starts first: input tiles have no deps, so
            # both DGE rings fill their descriptor queues up front and stream
            # ahead of the chain. (Interleaving them with the per-block ACT
            # ops would serialize each ring's next input chunk behind the
            # previous block's activation - the sequencers are in-order.)
            xts = []
            t0 = 0
            for bi, tb in enumerate(blocks):
                xt = xpool.tile([P, tb * F], mybir.dt.float32, tag="xin")
                # split EVERY block across both HW-DGE rings: each block
                # lands in half the time and the early blocks never gate
                # the chain (ring rate drops to ~170 GB/s once the DVE is
                # streaming, so a single ring can fall behind at the start)
                if tb == 1:
                    hp = P // 2
                    nc.sync.dma_start(xt[:hp, :], x[:hp, t0:t0 + 1, :])
                    nc.scalar.dma_start(xt[hp:, :], x[hp:, t0:t0 + 1, :])
                else:
                    th = (tb + 1) // 2
                    nc.sync.dma_start(xt[:, :th * F], x[:, t0:t0 + th, :])
                    nc.scalar.dma_start(xt[:, th * F:], x[:, t0 + th:t0 + tb, :])
                xts.append(xt)
                t0 += tb
            t0 = 0
            for bi, tb in enumerate(blocks):
                xt = xts[bi]
                ub = ubpool.tile([P, tb * F], mybir.dt.float32, tag="ub")
                for ti in range(tb):
                    if t0 + ti == 0:
                        # v_0 = 0, so u_0 = x_0: skip the add, the reset and
                        # the spike pass read the x tile directly
                        for h in range(2):
                            nc.vector.scalar_tensor_tensor(
                                v[:, h * H:(h + 1) * H], xt[:, h * H:h * H + H],
                                V_TH, xt[:, h * H:h * H + H],
                                mybir.AluOpType.is_lt, mybir.AluOpType.mult,
                            )
                        continue
                    for h in range(2):
                        lo = ti * F + h * H
                        nc.vector.tensor_add(
                            ub[:, lo:lo + H], v[:, h * H:(h + 1) * H],
                            xt[:, lo:lo + H],
                        )
                    if t0 + ti == T - 1:
                        continue  # v after the final timestep is never read
                    for h in range(2):
                        lo = ti * F + h * H
                        nc.vector.scalar_tensor_tensor(
                            v[:, h * H:(h + 1) * H], ub[:, lo:lo + H], V_TH,
                            ub[:, lo:lo + H],
                            mybir.AluOpType.is_lt, mybir.AluOpType.mult,
                        )
                st = spool.tile([P, tb * F], mybir.dt.uint8, tag="sout")
                usrc = xt if t0 == 0 and tb == 1 else ub
                if t0 + tb > T - 3:
                    # final blocks: the ACT hop (engine handoff + 222-cycle
                    # SBUF latency) is a pure tail; one DVE op computes
                    # z = (u < V_TH) directly instead
                    nc.vector.tensor_scalar(
                        st[:], usrc[:], V_TH, None, mybir.AluOpType.is_lt
                    )
                else:
                    # One ACT pass: z = Sign(V_TH - u) in {-1,0,1}; the uint8
                    # store saturates to {0,1}, so z = [u < V_TH] = 1 - s
                    # exactly (u == V_TH -> Sign(0) = 0 -> spike). Host flips.
                    nc.scalar.activation(
                        st[:], usrc[:], mybir.ActivationFunctionType.Sign,
                        bias=V_TH, scale=-1.0,
                    )
                # outputs alternate rings; they enqueue behind that ring's
                # input chunks (FIFO), which are long done by then
                oeng = nc.sync if bi % 2 == 0 else nc.scalar
                oeng.dma_start(y[:, t0:t0 + tb, :], st[:])
                t0 += tb
    nc.compile()
    return nc


def _get_nc():
    if "nc" not in _NC_CACHE:
        _NC_CACHE["nc"] = build_nc()
    return _NC_CACHE["nc"]


def run_sharded(x_seq, trace=False, nc=None, **kwargs):
    if nc is None:
        nc = _get_nc()
    x2 = np.asarray(x_seq, dtype=np.float32).reshape(T, B * N)
    in_maps = []
    for c in range(NCORES):
        # core slab [T, PERCORE] -> [P, T, F]: partition-major, time
        # contiguous per partition so each block is one fat descriptor
        xc = x2[:, c * PERCORE:(c + 1) * PERCORE].reshape(T, P, F)
        in_maps.append({"x": np.ascontiguousarray(xc.transpose(1, 0, 2))})
    # A cold device occasionally reports NRT_EXEC_UNIT_UNRECOVERABLE on the
    # first execute and recovers on the next attempt; retry a couple times.
    for attempt in range(3):
        try:
            res = run_bass_kernel_spmd(
                nc, in_maps, list(range(NCORES)), trace=trace, **kwargs
            )
            break
        except Exception:  # jax.errors.JaxRuntimeError and friends
            if attempt == 2:
                raise
            import time
            time.sleep(2.0)
    out = np.empty((T, B * N), dtype=np.float32)
    for c in range(NCORES):
        zc = np.asarray(res.results[c]["y"])          # [P, T, F] uint8, z = 1-s
        r = zc.transpose(1, 0, 2).reshape(T, PERCORE)
        out[:, c * PERCORE:(c + 1) * PERCORE] = 1 - r
    return out.reshape(T, B, N), res


def kernel(x_seq):
    out, _ = run_sharded(x_seq)
    return out
